# revision 29
# baseline (speedup 1.0000x reference)
"""Distributed inverse real vector SHT on 8 Trainium2 NeuronCores.

Decomposition (2D polar x azimuth, per the original model's parallelism):
  Stage 1 (sharded over m): for each m, the four Legendre contractions are
  two accumulating matmuls  Z[m] = X1[m]^T @ dT0[m] + X2[m]^T @ dT1[m]
  where the 128 columns of X1/X2 pack the four (re/im x s/t) input blocks
  with signs arranged so the PSUM accumulation directly produces
  rows [srl, sim, tim, trl].  Only X1 is loaded from HBM; X2 is a signed
  block-permutation of X1 built on-chip (halves the x traffic).
  All-to-all (3 segments along m, fired as stage 1 completes each).
  Stage 2 (sharded over k): per segment chunk, transpose Z on the PE
  (c2 partition -> m partition), then run the irfft matmuls against
  precomputed cos/sin tables; chunks accumulate into an SBUF f32
  accumulator as their collectives land, so only the last chunk's
  matmuls and a fused add-evict trail the final collective.
"""
import sys
import os
sys.path.insert(0, '/opt/trn_rl_repo')
import numpy as np
import ml_dtypes

from concourse import bacc, tile, mybir, masks
from concourse.bass_utils import run_bass_kernel_spmd

B, C, L, M, K, N = 1, 32, 361, 361, 361, 720
NC = 8
MP = 368                    # m padded to 8*46
MC = MP // NC               # 46 m's per core
MSEG = [(0, 14), (14, 16), (30, 16)]   # 3 a2a segments = stage-2 chunks
KC = 46                     # k's per core
KPP = 368
LP = 384                    # l padded to 3*128
LCH = 3
NG = (KC + 3) // 4          # 12 kj-groups (last has 2)
BF16 = ml_dtypes.bfloat16
ACT_COPY = mybir.ActivationFunctionType.Copy

_CACHE = {}


def _build():
    nc = bacc.Bacc("TRN2", target_bir_lowering=False, debug=False,
                   num_devices=NC)
    xsh = nc.dram_tensor("xsh", [128, MC, LCH, 128], mybir.dt.bfloat16,
                         kind="ExternalInput")
    dsh = nc.dram_tensor("dsh", [128, MC, 2, LCH, KPP], mybir.dt.bfloat16,
                         kind="ExternalInput")
    # butterfly tables: slot 0 = w*cos, 1 = -w*sin, over n' = 0..367
    # (out[n'] = E+S, out[720-n'] = E-S)
    ctab = nc.dram_tensor("ctab", [128, 2, LCH, 368], mybir.dt.bfloat16,
                          kind="ExternalInput")
    outsh = nc.dram_tensor("outsh", [2, 2, 3, 128, 1472], mybir.dt.bfloat16,
                           kind="ExternalOutput")

    m_blocks = [(0, 8), (8, 6), (14, 8), (22, 8), (30, 8), (38, 8)]

    with tile.TileContext(nc) as tc:
        with tc.tile_pool(name="dram", bufs=1, space="DRAM") as dram, \
             tc.tile_pool(name="const", bufs=1) as constp:
            a2a_in = [dram.tile([NC, 128, mw, KC], mybir.dt.bfloat16,
                                name=f"a2ain{s}")
                      for s, (ms, mw) in enumerate(MSEG)]
            a2a_out = [dram.tile([NC, 128, mw, KC], mybir.dt.bfloat16,
                                 name=f"a2aout{s}")
                       for s, (ms, mw) in enumerate(MSEG)]
            ident = constp.tile([128, 128], mybir.dt.bfloat16)
            masks.make_identity(nc, ident[:])

            # ---------------- stage 1: Legendre contractions (m-sharded)
            with tc.tile_pool(name="s1", bufs=2) as s1, \
                 tc.tile_pool(name="zs", bufs=1) as zs, \
                 tc.tile_pool(name="ps1", bufs=4, space="PSUM") as ps1:
                zst = [zs.tile([128, NC, mw, KC], mybir.dt.bfloat16,
                               name=f"zst{s}")
                       for s, (ms, mw) in enumerate(MSEG)]
                for (m0, cnt) in m_blocks:
                    dt = s1.tile([128, 8, 2, LCH, KPP], mybir.dt.bfloat16,
                                 tag="dt")
                    x1 = s1.tile([128, 8, LCH, 128], mybir.dt.bfloat16,
                                 tag="x1")
                    x2 = s1.tile([128, 8, LCH, 128], mybir.dt.bfloat16,
                                 tag="x2")
                    nc.sync.dma_start(out=dt[:, :cnt], in_=dsh[:, m0:m0+cnt])
                    nc.sync.dma_start(out=x1[:, :cnt], in_=xsh[:, m0:m0+cnt])
                    # X2 = [X1 blk2, -X1 blk3, X1 blk0, -X1 blk1]
                    nc.scalar.activation(out=x2[:, :cnt, :, 0:32],
                                         in_=x1[:, :cnt, :, 64:96],
                                         func=ACT_COPY)
                    nc.vector.tensor_scalar_mul(x2[:, :cnt, :, 32:64],
                                                x1[:, :cnt, :, 96:128], -1.0)
                    nc.scalar.activation(out=x2[:, :cnt, :, 64:96],
                                         in_=x1[:, :cnt, :, 0:32],
                                         func=ACT_COPY)
                    nc.vector.tensor_scalar_mul(x2[:, :cnt, :, 96:128],
                                                x1[:, :cnt, :, 32:64], -1.0)
                    xw = (x1, x2)
                    for ml in range(cnt):
                        zt = ps1.tile([128, KPP], mybir.dt.float32, tag="zt")
                        for lc in range(LCH):
                            for w in range(2):
                                nc.tensor.matmul(
                                    out=zt[:],
                                    lhsT=xw[w][:, ml, lc, :],
                                    rhs=dt[:, ml, w, lc, :],
                                    start=(lc == 0 and w == 0),
                                    stop=(lc == LCH - 1 and w == 1),
                                )
                        mg = m0 + ml
                        seg = next(s for s, (ms, mw) in enumerate(MSEG)
                                   if ms <= mg < ms + mw)
                        zv = zt[:].rearrange("p (g k) -> p g k", k=KC)
                        dst = zst[seg][:, :, mg - MSEG[seg][0], :]
                        if mg % 2 == 0:
                            nc.vector.tensor_copy(out=dst, in_=zv)
                        else:
                            nc.scalar.activation(out=dst, in_=zv,
                                                 func=ACT_COPY)
                    # fire each segment's a2a writes as soon as complete
                    # (seg 2 rides the sync ring, idle once input is done)
                    for s, (ms, mw) in enumerate(MSEG):
                        if m0 + cnt == ms + mw:
                            weng = nc.sync if s == 2 else nc.scalar
                            for kg in range(NC):
                                weng.dma_start(
                                    out=a2a_in[s][kg], in_=zst[s][:, kg])

            for s in range(len(MSEG)):
                nc.gpsimd.collective_compute(
                    "AllToAll", mybir.AluOpType.bypass,
                    replica_groups=[list(range(NC))],
                    ins=[a2a_in[s].opt()], outs=[a2a_out[s].opt()],
                )

            # ---------------- stage 2: irfft as matmul (k-sharded)
            # Butterfly: E = Cre^T zre, S = Cim^T zim over n' = 0..360;
            # out[n'] = E + S, out[720-n'] = E - S.
            # comp 0 (s): E from srl rows (b=0), S from sim rows (b=1)
            # comp 1 (t): E from trl rows (b=3), S from tim rows (b=2)
            comp_es = [(0, 1), (3, 2)]
            with tc.tile_pool(name="s2", bufs=1) as s2, \
                 tc.tile_pool(name="zp", bufs=2) as zp, \
                 tc.tile_pool(name="zr", bufs=2) as zr, \
                 tc.tile_pool(name="ob", bufs=4) as ob, \
                 tc.tile_pool(name="ps2tp", bufs=2, space="PSUM") as ps2tp, \
                 tc.tile_pool(name="ps2po", bufs=2, space="PSUM") as ps2po:
                ct = s2.tile([128, 2, LCH, 368], mybir.dt.bfloat16, tag="ct")
                nc.sync.dma_start(out=ct[:], in_=ctab[:])
                # bf16 accumulators: [n-part, comp, E/S, nu, kc]
                acc = s2.tile([128, 2, 2, 3, 1472], mybir.dt.bfloat16,
                              tag="acc")

                for ci, (ms, mw) in enumerate(MSEG):
                    mcnt = NC * mw
                    ztmp = zp.tile([128, NC, mw, KC], mybir.dt.bfloat16,
                                   name=f"ztmp{ci}")
                    nc.sync.dma_start(
                        out=ztmp[:],
                        in_=a2a_out[ci].rearrange("b c m k -> c b m k"))
                    zsrc = ztmp.rearrange("c b m k -> c (b m) k")

                    # transpose c2->m via matmuls against the identity;
                    # 4 transposes per psum bank, one wide eviction copy
                    t = zr.tile([128, 4, NG, 4, 32], mybir.dt.bfloat16,
                                tag="ztr")
                    nc.vector.memset(t[:, :, NG-1, 2:4, :], 0.0)
                    for g in range(NG):
                        kw = min(4, KC - g * 4)
                        ptb = ps2tp.tile([128, 512], mybir.dt.float32,
                                         tag="ptb")
                        for kk in range(kw):
                            nc.tensor.matmul(
                                out=ptb[:mcnt, kk*128:(kk+1)*128],
                                lhsT=zsrc[:, :mcnt, g*4 + kk],
                                rhs=ident[:], start=True, stop=True)
                        pv = ptb[:mcnt, 0:kw*128].rearrange(
                            "m (k b c) -> m b k c", b=4, c=32)
                        if g % 2 == 0:
                            nc.vector.tensor_copy(out=t[:mcnt, :, g, 0:kw, :],
                                                  in_=pv)
                        else:
                            nc.scalar.activation(out=t[:mcnt, :, g, 0:kw, :],
                                                 in_=pv, func=ACT_COPY)

                    # operand-swapped irfft: C-table chunks are the
                    # stationary weights (their 128-row LDWEIGHTS hides
                    # under 512-col z streams); output is n-partitioned.
                    tv = t.rearrange("p b g kk c -> p b (g kk c)")
                    for comp in range(2):
                        bbE, bbS = comp_es[comp]
                        for nu in range(3):
                            pw = min(128, 368 - nu * 128)
                            nsl = slice(nu * 128, nu * 128 + pw)
                            # every chunk: one E pass + one S pass
                            potE = ps2po.tile([128, 1536],
                                              mybir.dt.float32,
                                              tag="pot")
                            potS = ps2po.tile([128, 1536],
                                              mybir.dt.float32,
                                              tag="pot")
                            for pot, reim, bb in ((potE, 0, bbE),
                                                  (potS, 1, bbS)):
                                for r0 in (0, 512, 1024):
                                    r1 = min(r0 + 512, 1472)
                                    nc.tensor.matmul(
                                        out=pot[:pw, r0:r1],
                                        lhsT=ct[:mcnt, reim, ci, nsl],
                                        rhs=tv[:mcnt, bb, r0:r1],
                                        start=True, stop=True)
                            aE = acc[:, comp, 0, nu]
                            aS = acc[:, comp, 1, nu]
                            if ci == 0:
                                nc.vector.tensor_copy(
                                    out=aE[:pw, 0:736],
                                    in_=potE[:pw, 0:736])
                                nc.scalar.activation(
                                    out=aE[:pw, 736:1472],
                                    in_=potE[:pw, 736:1472],
                                    func=ACT_COPY)
                                nc.vector.tensor_copy(
                                    out=aS[:pw, 0:736],
                                    in_=potS[:pw, 0:736])
                                nc.scalar.activation(
                                    out=aS[:pw, 736:1472],
                                    in_=potS[:pw, 736:1472],
                                    func=ACT_COPY)
                            else:
                                nc.vector.tensor_tensor(
                                    aE[:pw], potE[:pw, 0:1472],
                                    aE[:pw], mybir.AluOpType.add)
                                nc.vector.tensor_tensor(
                                    aS[:pw], potS[:pw, 0:1472],
                                    aS[:pw], mybir.AluOpType.add)
                            if ci == 2:
                                # butterfly combine rides DVE (lo) and
                                # gpsimd (hi) while the PE streams the
                                # next (comp, nu) pair
                                osbL = ob.tile([128, 1472],
                                               mybir.dt.bfloat16,
                                               tag="osb")
                                osbH = ob.tile([128, 1472],
                                               mybir.dt.bfloat16,
                                               tag="osb")
                                nc.vector.tensor_tensor(
                                    osbL[:pw], aE[:pw], aS[:pw],
                                    mybir.AluOpType.add)
                                nc.gpsimd.tensor_tensor(
                                    osbH[:pw], aE[:pw], aS[:pw],
                                    mybir.AluOpType.subtract)
                                deng = (nc.sync if nu % 2 == 0
                                        else nc.scalar)
                                deng.dma_start(
                                    out=outsh[0, comp, nu][0:pw],
                                    in_=osbL[:pw])
                                deng = (nc.sync if nu % 2 == 1
                                        else nc.scalar)
                                deng.dma_start(
                                    out=outsh[1, comp, nu][0:pw],
                                    in_=osbH[:pw])
    nc.compile()
    return nc


def _m_perm():
    """Row order of the m axis as seen by stage 2 (segment-major)."""
    perm = []
    for (ms, mw) in MSEG:
        perm += [mb * MC + ms + ml for mb in range(NC) for ml in range(mw)]
    return np.array(perm)


def _host_prep(x_re, x_im, d0, d1):
    xr0, xr1 = x_re[0, :, 0], x_re[0, :, 1]   # (32, L, M)
    xi0, xi1 = x_im[0, :, 0], x_im[0, :, 1]

    # X1 pack only; X2 is built on-chip from X1
    x = np.concatenate([xr0, xi0, -xi1, -xr1], axis=0)   # (128, L, M)
    x = np.transpose(x, (2, 1, 0))                       # (M, L, 128)
    xp = np.zeros((MP, LP, 128), BF16)
    xp[:M, :L] = x
    xv = xp.reshape(NC, MC, LCH, 128, 128)               # (i, ml, lc, p, c)
    xv = np.ascontiguousarray(xv.transpose(0, 3, 1, 2, 4))

    def mkd(d):
        dp = np.zeros((MP, LP, KPP), BF16)
        dp[:M, :L, :K] = np.transpose(d, (0, 2, 1))
        return dp
    D0, D1 = mkd(d0), mkd(d1)
    dv = np.stack([D0, D1], axis=1)                   # (MP, 2, LP, KPP)
    dv = dv.reshape(NC, MC, 2, LCH, 128, KPP)
    dv = np.ascontiguousarray(dv.transpose(0, 4, 1, 2, 3, 5))

    m = np.arange(MP, dtype=np.float64)[:, None]
    n = np.arange(368, dtype=np.float64)[None, :]
    th = 2.0 * np.pi * (m * n) / N
    w = np.full((MP, 1), 2.0); w[0] = 1.0; w[360] = 1.0; w[361:] = 0.0
    Cre = (w * np.cos(th)).astype(np.float32)
    Cim = (-w * np.sin(th)).astype(np.float32)
    Cim[0] = 0.0; Cim[360] = 0.0; Cim[361:] = 0.0
    cv = np.stack([Cre, Cim], axis=1)                 # (MP, 2, 368)
    # chunk-aligned layout: each segment's rows start at a 128-row slot
    cvc = np.zeros((LCH, 128, 2, 368), np.float32)
    for s, (ms, mw) in enumerate(MSEG):
        rows = np.array([mb * MC + ms + ml
                         for mb in range(NC) for ml in range(mw)])
        cvc[s, :len(rows)] = cv[rows]
    cv = np.ascontiguousarray(cvc.transpose(1, 2, 0, 3)).astype(BF16)
    return xv, dv, cv


def kernel(x_re, x_im, d0, d1):
    if "nc" not in _CACHE:
        _CACHE["nc"] = _build()
    nc = _CACHE["nc"]

    xv, dv, cv = _host_prep(np.asarray(x_re), np.asarray(x_im),
                            np.asarray(d0), np.asarray(d1))
    in_maps = [{"xsh": xv[i], "dsh": dv[i], "ctab": cv} for i in range(NC)]
    res = run_bass_kernel_spmd(nc, in_maps, list(range(NC)))

    out = np.empty((B, C, 2, K, N), np.float32)
    for i in range(NC):
        k0 = i * KC
        kv = min(K, k0 + KC) - k0
        # [lohi, comp, nu, p, kc] -> [c, comp, k, lohi, n'=nu*128+p]
        o = res.results[i]["outsh"].astype(np.float32)
        o = o.reshape(2, 2, 3, 128, KC, C)
        o = o.transpose(5, 1, 4, 0, 2, 3).reshape(C, 2, KC, 2, 384)
        lo = o[:, :, :kv, 0, :]
        hi = o[:, :, :kv, 1, :]
        out[0, :, :, k0:k0+kv, 0:361] = lo[..., 0:361]
        out[0, :, :, k0:k0+kv, 361:720] = hi[..., 1:360][..., ::-1]
    return out



# revision 30
# speedup vs baseline: 1.2209x; 1.2209x over previous
"""Distributed inverse real vector SHT on 8 Trainium2 NeuronCores.

Decomposition (2D polar x azimuth, per the original model's parallelism):
  Stage 1 (sharded over m): for each m, the four Legendre contractions are
  two accumulating matmuls  Z[m] = X1[m]^T @ dT0[m] + X2[m]^T @ dT1[m]
  where the 128 columns of X1/X2 pack the four (re/im x s/t) input blocks
  with signs arranged so the PSUM accumulation directly produces
  rows [srl, sim, tim, trl].  Only X1 is loaded from HBM; X2 is a signed
  block-permutation of X1 built on-chip (halves the x traffic).
  All-to-all (3 segments along m, fired as stage 1 completes each).
  Stage 2 (sharded over k): per segment chunk, transpose Z on the PE
  (c2 partition -> m partition), then run the irfft matmuls against
  precomputed cos/sin tables; chunks accumulate into an SBUF f32
  accumulator as their collectives land, so only the last chunk's
  matmuls and a fused add-evict trail the final collective.
"""
import sys
import os
sys.path.insert(0, '/opt/trn_rl_repo')
import numpy as np
import ml_dtypes

from concourse import bacc, tile, mybir, masks
from concourse.bass_utils import run_bass_kernel_spmd

B, C, L, M, K, N = 1, 32, 361, 361, 361, 720
NC = 8
MP = 368                    # m padded to 8*46
MC = MP // NC               # 46 m's per core
MSEG = [(0, 14), (14, 16), (30, 16)]   # 3 a2a segments = stage-2 chunks
KC = 46                     # k's per core
KPP = 368
LP = 384                    # l padded to 3*128
LCH = 3
NG = (KC + 3) // 4          # 12 kj-groups (last has 2)
BF16 = ml_dtypes.bfloat16
ACT_COPY = mybir.ActivationFunctionType.Copy

_CACHE = {}


def _build():
    nc = bacc.Bacc("TRN2", target_bir_lowering=False, debug=False,
                   num_devices=NC)
    xsh = nc.dram_tensor("xsh", [128, MC, LCH, 128], mybir.dt.bfloat16,
                         kind="ExternalInput")
    dsh = nc.dram_tensor("dsh", [128, MC, 2, LCH, KPP], mybir.dt.bfloat16,
                         kind="ExternalInput")
    # butterfly tables: slot 0 = w*cos, 1 = -w*sin, over n' = 0..367
    # (out[n'] = E+S, out[720-n'] = E-S)
    ctab = nc.dram_tensor("ctab", [128, 2, LCH, 368], mybir.dt.bfloat16,
                          kind="ExternalInput")
    outsh = nc.dram_tensor("outsh", [2, 2, 3, 128, 1472], mybir.dt.bfloat16,
                           kind="ExternalOutput")

    m_blocks = [(0, 8), (8, 6), (14, 8), (22, 8), (30, 8), (38, 8)]

    with tile.TileContext(nc) as tc:
        with tc.tile_pool(name="dram", bufs=1, space="DRAM") as dram, \
             tc.tile_pool(name="const", bufs=1) as constp:
            a2a_in = [dram.tile([NC, 128, mw, KC], mybir.dt.bfloat16,
                                name=f"a2ain{s}")
                      for s, (ms, mw) in enumerate(MSEG)]
            a2a_out = [dram.tile([NC, 128, mw, KC], mybir.dt.bfloat16,
                                 name=f"a2aout{s}")
                       for s, (ms, mw) in enumerate(MSEG)]
            ident = constp.tile([128, 128], mybir.dt.bfloat16)
            masks.make_identity(nc, ident[:])

            # ---------------- stage 1: Legendre contractions (m-sharded)
            with tc.tile_pool(name="s1", bufs=2) as s1, \
                 tc.tile_pool(name="zs", bufs=1) as zs, \
                 tc.tile_pool(name="ps1", bufs=4, space="PSUM") as ps1:
                zst = [zs.tile([128, NC, mw, KC], mybir.dt.bfloat16,
                               name=f"zst{s}")
                       for s, (ms, mw) in enumerate(MSEG)]
                for (m0, cnt) in m_blocks:
                    dt = s1.tile([128, 8, 2, LCH, KPP], mybir.dt.bfloat16,
                                 tag="dt")
                    x1 = s1.tile([128, 8, LCH, 128], mybir.dt.bfloat16,
                                 tag="x1")
                    x2 = s1.tile([128, 8, LCH, 128], mybir.dt.bfloat16,
                                 tag="x2")
                    nc.sync.dma_start(out=dt[:, :cnt], in_=dsh[:, m0:m0+cnt])
                    nc.sync.dma_start(out=x1[:, :cnt], in_=xsh[:, m0:m0+cnt])
                    # X2 = [X1 blk2, -X1 blk3, X1 blk0, -X1 blk1]
                    nc.scalar.activation(out=x2[:, :cnt, :, 0:32],
                                         in_=x1[:, :cnt, :, 64:96],
                                         func=ACT_COPY)
                    nc.vector.tensor_scalar_mul(x2[:, :cnt, :, 32:64],
                                                x1[:, :cnt, :, 96:128], -1.0)
                    nc.scalar.activation(out=x2[:, :cnt, :, 64:96],
                                         in_=x1[:, :cnt, :, 0:32],
                                         func=ACT_COPY)
                    nc.vector.tensor_scalar_mul(x2[:, :cnt, :, 96:128],
                                                x1[:, :cnt, :, 32:64], -1.0)
                    xw = (x1, x2)
                    for ml in range(cnt):
                        zt = ps1.tile([128, KPP], mybir.dt.float32, tag="zt")
                        for lc in range(LCH):
                            for w in range(2):
                                nc.tensor.matmul(
                                    out=zt[:],
                                    lhsT=xw[w][:, ml, lc, :],
                                    rhs=dt[:, ml, w, lc, :],
                                    start=(lc == 0 and w == 0),
                                    stop=(lc == LCH - 1 and w == 1),
                                )
                        mg = m0 + ml
                        seg = next(s for s, (ms, mw) in enumerate(MSEG)
                                   if ms <= mg < ms + mw)
                        zv = zt[:].rearrange("p (g k) -> p g k", k=KC)
                        dst = zst[seg][:, :, mg - MSEG[seg][0], :]
                        if mg % 2 == 0:
                            nc.vector.tensor_copy(out=dst, in_=zv)
                        else:
                            nc.scalar.activation(out=dst, in_=zv,
                                                 func=ACT_COPY)
                    # fire each segment's a2a writes as soon as complete
                    # (seg 2 rides the sync ring, idle once input is done)
                    for s, (ms, mw) in enumerate(MSEG):
                        if m0 + cnt == ms + mw:
                            weng = nc.sync if s == 2 else nc.scalar
                            for kg in range(NC):
                                weng.dma_start(
                                    out=a2a_in[s][kg], in_=zst[s][:, kg])

            for s in range(len(MSEG)):
                nc.gpsimd.collective_compute(
                    "AllToAll", mybir.AluOpType.bypass,
                    replica_groups=[list(range(NC))],
                    ins=[a2a_in[s].opt()], outs=[a2a_out[s].opt()],
                )

            # ---------------- stage 2: irfft as matmul (k-sharded)
            # Butterfly: E = Cre^T zre, S = Cim^T zim over n' = 0..360;
            # out[n'] = E + S, out[720-n'] = E - S.
            # comp 0 (s): E from srl rows (b=0), S from sim rows (b=1)
            # comp 1 (t): E from trl rows (b=3), S from tim rows (b=2)
            comp_es = [(0, 1), (3, 2)]
            with tc.tile_pool(name="s2", bufs=1) as s2, \
                 tc.tile_pool(name="zp", bufs=1) as zp, \
                 tc.tile_pool(name="zr", bufs=2) as zr, \
                 tc.tile_pool(name="ob", bufs=4) as ob, \
                 tc.tile_pool(name="ps2tp", bufs=2, space="PSUM") as ps2tp, \
                 tc.tile_pool(name="ps2po", bufs=2, space="PSUM") as ps2po:
                ct = s2.tile([128, 2, LCH, 368], mybir.dt.bfloat16, tag="ct")
                nc.sync.dma_start(out=ct[:], in_=ctab[:])
                # bf16 accumulators: [n-part, comp, E/S, nu, kc]
                acc = s2.tile([128, 2, 2, 3, 1472], mybir.dt.bfloat16,
                              tag="acc")

                for ci, (ms, mw) in enumerate(MSEG):
                    mcnt = NC * mw
                    ztmp = zp.tile([128, NC, mw, KC], mybir.dt.bfloat16,
                                   name=f"ztmp{ci}")
                    nc.scalar.dma_start(
                        out=ztmp[:],
                        in_=a2a_out[ci].rearrange("b c m k -> c b m k"))
                    zsrc = ztmp.rearrange("c b m k -> c (b m) k")

                    # transpose c2->m via matmuls against the identity;
                    # 4 transposes per psum bank, one wide eviction copy
                    t = zr.tile([128, 4, NG, 4, 32], mybir.dt.bfloat16,
                                tag="ztr")
                    nc.vector.memset(t[:, :, NG-1, 2:4, :], 0.0)
                    for g in range(NG):
                        kw = min(4, KC - g * 4)
                        ptb = ps2tp.tile([128, 512], mybir.dt.float32,
                                         tag="ptb")
                        for kk in range(kw):
                            nc.tensor.matmul(
                                out=ptb[:mcnt, kk*128:(kk+1)*128],
                                lhsT=zsrc[:, :mcnt, g*4 + kk],
                                rhs=ident[:], start=True, stop=True)
                        pv = ptb[:mcnt, 0:kw*128].rearrange(
                            "m (k b c) -> m b k c", b=4, c=32)
                        if g % 2 == 0:
                            nc.vector.tensor_copy(out=t[:mcnt, :, g, 0:kw, :],
                                                  in_=pv)
                        else:
                            nc.scalar.activation(out=t[:mcnt, :, g, 0:kw, :],
                                                 in_=pv, func=ACT_COPY)

                    # operand-swapped irfft: C-table chunks are the
                    # stationary weights (their 128-row LDWEIGHTS hides
                    # under 512-col z streams); output is n-partitioned.
                    tv = t.rearrange("p b g kk c -> p b (g kk c)")
                    for comp in range(2):
                        bbE, bbS = comp_es[comp]
                        for nu in range(3):
                            pw = min(128, 368 - nu * 128)
                            nsl = slice(nu * 128, nu * 128 + pw)
                            # every chunk: one E pass + one S pass
                            potE = ps2po.tile([128, 1536],
                                              mybir.dt.float32,
                                              tag="pot")
                            potS = ps2po.tile([128, 1536],
                                              mybir.dt.float32,
                                              tag="pot")
                            for pot, reim, bb in ((potE, 0, bbE),
                                                  (potS, 1, bbS)):
                                for r0 in (0, 512, 1024):
                                    r1 = min(r0 + 512, 1472)
                                    nc.tensor.matmul(
                                        out=pot[:pw, r0:r1],
                                        lhsT=ct[:mcnt, reim, ci, nsl],
                                        rhs=tv[:mcnt, bb, r0:r1],
                                        start=True, stop=True)
                            aE = acc[:, comp, 0, nu]
                            aS = acc[:, comp, 1, nu]
                            if ci == 0:
                                nc.vector.tensor_copy(
                                    out=aE[:pw, 0:736],
                                    in_=potE[:pw, 0:736])
                                nc.scalar.activation(
                                    out=aE[:pw, 736:1472],
                                    in_=potE[:pw, 736:1472],
                                    func=ACT_COPY)
                                nc.vector.tensor_copy(
                                    out=aS[:pw, 0:736],
                                    in_=potS[:pw, 0:736])
                                nc.scalar.activation(
                                    out=aS[:pw, 736:1472],
                                    in_=potS[:pw, 736:1472],
                                    func=ACT_COPY)
                            else:
                                nc.vector.tensor_tensor(
                                    aE[:pw], potE[:pw, 0:1472],
                                    aE[:pw], mybir.AluOpType.add)
                                nc.vector.tensor_tensor(
                                    aS[:pw], potS[:pw, 0:1472],
                                    aS[:pw], mybir.AluOpType.add)
                            if ci == 2:
                                # butterfly combine rides DVE (lo) and
                                # gpsimd (hi) while the PE streams the
                                # next (comp, nu) pair
                                osbL = ob.tile([128, 1472],
                                               mybir.dt.bfloat16,
                                               tag="osb")
                                osbH = ob.tile([128, 1472],
                                               mybir.dt.bfloat16,
                                               tag="osb")
                                nc.vector.tensor_tensor(
                                    osbL[:pw], aE[:pw], aS[:pw],
                                    mybir.AluOpType.add)
                                nc.gpsimd.tensor_tensor(
                                    osbH[:pw], aE[:pw], aS[:pw],
                                    mybir.AluOpType.subtract)
                                deng = (nc.sync if nu % 2 == 0
                                        else nc.scalar)
                                deng.dma_start(
                                    out=outsh[0, comp, nu][0:pw],
                                    in_=osbL[:pw])
                                deng = (nc.sync if nu % 2 == 1
                                        else nc.scalar)
                                deng.dma_start(
                                    out=outsh[1, comp, nu][0:pw],
                                    in_=osbH[:pw])
    nc.compile()
    return nc


def _m_perm():
    """Row order of the m axis as seen by stage 2 (segment-major)."""
    perm = []
    for (ms, mw) in MSEG:
        perm += [mb * MC + ms + ml for mb in range(NC) for ml in range(mw)]
    return np.array(perm)


def _host_prep(x_re, x_im, d0, d1):
    xr0, xr1 = x_re[0, :, 0], x_re[0, :, 1]   # (32, L, M)
    xi0, xi1 = x_im[0, :, 0], x_im[0, :, 1]

    # X1 pack only; X2 is built on-chip from X1
    x = np.concatenate([xr0, xi0, -xi1, -xr1], axis=0)   # (128, L, M)
    x = np.transpose(x, (2, 1, 0))                       # (M, L, 128)
    xp = np.zeros((MP, LP, 128), BF16)
    xp[:M, :L] = x
    xv = xp.reshape(NC, MC, LCH, 128, 128)               # (i, ml, lc, p, c)
    xv = np.ascontiguousarray(xv.transpose(0, 3, 1, 2, 4))

    def mkd(d):
        dp = np.zeros((MP, LP, KPP), BF16)
        dp[:M, :L, :K] = np.transpose(d, (0, 2, 1))
        return dp
    D0, D1 = mkd(d0), mkd(d1)
    dv = np.stack([D0, D1], axis=1)                   # (MP, 2, LP, KPP)
    dv = dv.reshape(NC, MC, 2, LCH, 128, KPP)
    dv = np.ascontiguousarray(dv.transpose(0, 4, 1, 2, 3, 5))

    m = np.arange(MP, dtype=np.float64)[:, None]
    n = np.arange(368, dtype=np.float64)[None, :]
    th = 2.0 * np.pi * (m * n) / N
    w = np.full((MP, 1), 2.0); w[0] = 1.0; w[360] = 1.0; w[361:] = 0.0
    Cre = (w * np.cos(th)).astype(np.float32)
    Cim = (-w * np.sin(th)).astype(np.float32)
    Cim[0] = 0.0; Cim[360] = 0.0; Cim[361:] = 0.0
    cv = np.stack([Cre, Cim], axis=1)                 # (MP, 2, 368)
    # chunk-aligned layout: each segment's rows start at a 128-row slot
    cvc = np.zeros((LCH, 128, 2, 368), np.float32)
    for s, (ms, mw) in enumerate(MSEG):
        rows = np.array([mb * MC + ms + ml
                         for mb in range(NC) for ml in range(mw)])
        cvc[s, :len(rows)] = cv[rows]
    cv = np.ascontiguousarray(cvc.transpose(1, 2, 0, 3)).astype(BF16)
    return xv, dv, cv


def kernel(x_re, x_im, d0, d1):
    if "nc" not in _CACHE:
        _CACHE["nc"] = _build()
    nc = _CACHE["nc"]

    xv, dv, cv = _host_prep(np.asarray(x_re), np.asarray(x_im),
                            np.asarray(d0), np.asarray(d1))
    in_maps = [{"xsh": xv[i], "dsh": dv[i], "ctab": cv} for i in range(NC)]
    res = run_bass_kernel_spmd(nc, in_maps, list(range(NC)))

    out = np.empty((B, C, 2, K, N), np.float32)
    for i in range(NC):
        k0 = i * KC
        kv = min(K, k0 + KC) - k0
        # [lohi, comp, nu, p, kc] -> [c, comp, k, lohi, n'=nu*128+p]
        o = res.results[i]["outsh"].astype(np.float32)
        o = o.reshape(2, 2, 3, 128, KC, C)
        o = o.transpose(5, 1, 4, 0, 2, 3).reshape(C, 2, KC, 2, 384)
        lo = o[:, :, :kv, 0, :]
        hi = o[:, :, :kv, 1, :]
        out[0, :, :, k0:k0+kv, 0:361] = lo[..., 0:361]
        out[0, :, :, k0:k0+kv, 361:720] = hi[..., 1:360][..., ::-1]
    return out



# revision 31
# speedup vs baseline: 1.2873x; 1.0544x over previous
"""Distributed inverse real vector SHT on 8 Trainium2 NeuronCores.

Decomposition (2D polar x azimuth, per the original model's parallelism):
  Stage 1 (sharded over m): for each m, the four Legendre contractions are
  two accumulating matmuls  Z[m] = X1[m]^T @ dT0[m] + X2[m]^T @ dT1[m]
  where the 128 columns of X1/X2 pack the four (re/im x s/t) input blocks
  with signs arranged so the PSUM accumulation directly produces
  rows [srl, sim, tim, trl].  Only X1 is loaded from HBM; X2 is a signed
  block-permutation of X1 built on-chip (halves the x traffic).
  All-to-all (3 segments along m, fired as stage 1 completes each).
  Stage 2 (sharded over k): per segment chunk, transpose Z on the PE
  (c2 partition -> m partition), then run the irfft matmuls against
  precomputed cos/sin tables; chunks accumulate into an SBUF f32
  accumulator as their collectives land, so only the last chunk's
  matmuls and a fused add-evict trail the final collective.
"""
import sys
import os
sys.path.insert(0, '/opt/trn_rl_repo')
import numpy as np
import ml_dtypes

from concourse import bacc, tile, mybir, masks
from concourse.bass_utils import run_bass_kernel_spmd

B, C, L, M, K, N = 1, 32, 361, 361, 361, 720
NC = 8
MP = 368                    # m padded to 8*46
MC = MP // NC               # 46 m's per core
MSEG = [(0, 14), (14, 16), (30, 16)]   # 3 a2a segments = stage-2 chunks
KC = 46                     # k's per core
KPP = 368
LP = 384                    # l padded to 3*128
LCH = 3
NG = (KC + 3) // 4          # 12 kj-groups (last has 2)
BF16 = ml_dtypes.bfloat16
ACT_COPY = mybir.ActivationFunctionType.Copy

_CACHE = {}


def _build():
    nc = bacc.Bacc("TRN2", target_bir_lowering=False, debug=False,
                   num_devices=NC)
    xsh = nc.dram_tensor("xsh", [128, MC, LCH, 128], mybir.dt.bfloat16,
                         kind="ExternalInput")
    dsh = nc.dram_tensor("dsh", [128, MC, 2, LCH, KPP], mybir.dt.bfloat16,
                         kind="ExternalInput")
    # butterfly tables: slot 0 = w*cos, 1 = -w*sin, over n' = 0..367
    # (out[n'] = E+S, out[720-n'] = E-S)
    ctab = nc.dram_tensor("ctab", [128, 2, LCH, 368], mybir.dt.bfloat16,
                          kind="ExternalInput")
    outsh = nc.dram_tensor("outsh", [2, 2, 3, 128, 1472], mybir.dt.bfloat16,
                           kind="ExternalOutput")

    m_blocks = [(0, 4), (4, 4), (8, 6), (14, 8), (22, 8), (30, 8), (38, 8)]

    with tile.TileContext(nc) as tc:
        with tc.tile_pool(name="dram", bufs=1, space="DRAM") as dram, \
             tc.tile_pool(name="const", bufs=1) as constp:
            a2a_in = [dram.tile([NC, 128, mw, KC], mybir.dt.bfloat16,
                                name=f"a2ain{s}")
                      for s, (ms, mw) in enumerate(MSEG)]
            a2a_out = [dram.tile([NC, 128, mw, KC], mybir.dt.bfloat16,
                                 name=f"a2aout{s}")
                       for s, (ms, mw) in enumerate(MSEG)]
            ident = constp.tile([128, 128], mybir.dt.bfloat16)
            masks.make_identity(nc, ident[:])

            # ---------------- stage 1: Legendre contractions (m-sharded)
            with tc.tile_pool(name="s1", bufs=2) as s1, \
                 tc.tile_pool(name="zs", bufs=1) as zs, \
                 tc.tile_pool(name="ps1", bufs=4, space="PSUM") as ps1:
                zst = [zs.tile([128, NC, mw, KC], mybir.dt.bfloat16,
                               name=f"zst{s}")
                       for s, (ms, mw) in enumerate(MSEG)]
                for (m0, cnt) in m_blocks:
                    dt = s1.tile([128, 8, 2, LCH, KPP], mybir.dt.bfloat16,
                                 tag="dt")
                    x1 = s1.tile([128, 8, LCH, 128], mybir.dt.bfloat16,
                                 tag="x1")
                    x2 = s1.tile([128, 8, LCH, 128], mybir.dt.bfloat16,
                                 tag="x2")
                    nc.sync.dma_start(out=dt[:, :cnt], in_=dsh[:, m0:m0+cnt])
                    nc.sync.dma_start(out=x1[:, :cnt], in_=xsh[:, m0:m0+cnt])
                    # X2 = [X1 blk2, -X1 blk3, X1 blk0, -X1 blk1]
                    nc.scalar.activation(out=x2[:, :cnt, :, 0:32],
                                         in_=x1[:, :cnt, :, 64:96],
                                         func=ACT_COPY)
                    nc.vector.tensor_scalar_mul(x2[:, :cnt, :, 32:64],
                                                x1[:, :cnt, :, 96:128], -1.0)
                    nc.scalar.activation(out=x2[:, :cnt, :, 64:96],
                                         in_=x1[:, :cnt, :, 0:32],
                                         func=ACT_COPY)
                    nc.vector.tensor_scalar_mul(x2[:, :cnt, :, 96:128],
                                                x1[:, :cnt, :, 32:64], -1.0)
                    xw = (x1, x2)
                    for ml in range(cnt):
                        zt = ps1.tile([128, KPP], mybir.dt.float32, tag="zt")
                        for lc in range(LCH):
                            for w in range(2):
                                nc.tensor.matmul(
                                    out=zt[:],
                                    lhsT=xw[w][:, ml, lc, :],
                                    rhs=dt[:, ml, w, lc, :],
                                    start=(lc == 0 and w == 0),
                                    stop=(lc == LCH - 1 and w == 1),
                                )
                        mg = m0 + ml
                        seg = next(s for s, (ms, mw) in enumerate(MSEG)
                                   if ms <= mg < ms + mw)
                        zv = zt[:].rearrange("p (g k) -> p g k", k=KC)
                        dst = zst[seg][:, :, mg - MSEG[seg][0], :]
                        if mg % 2 == 0:
                            nc.vector.tensor_copy(out=dst, in_=zv)
                        else:
                            nc.scalar.activation(out=dst, in_=zv,
                                                 func=ACT_COPY)
                    # fire each segment's a2a writes as soon as complete
                    # (seg 2 rides the sync ring, idle once input is done;
                    # its first half pre-flushes a block early)
                    for s, (ms, mw) in enumerate(MSEG):
                        if s == 2 and m0 + cnt == ms + 8:
                            for kg in range(NC):
                                nc.sync.dma_start(
                                    out=a2a_in[s][kg][:, 0:8],
                                    in_=zst[s][:, kg, 0:8])
                        elif m0 + cnt == ms + mw:
                            weng = nc.sync if s == 2 else nc.scalar
                            lo = 8 if s == 2 else 0
                            for kg in range(NC):
                                weng.dma_start(
                                    out=a2a_in[s][kg][:, lo:],
                                    in_=zst[s][:, kg, lo:])

            for s in range(len(MSEG)):
                nc.gpsimd.collective_compute(
                    "AllToAll", mybir.AluOpType.bypass,
                    replica_groups=[list(range(NC))],
                    ins=[a2a_in[s].opt()], outs=[a2a_out[s].opt()],
                )

            # ---------------- stage 2: irfft as matmul (k-sharded)
            # Butterfly: E = Cre^T zre, S = Cim^T zim over n' = 0..360;
            # out[n'] = E + S, out[720-n'] = E - S.
            # comp 0 (s): E from srl rows (b=0), S from sim rows (b=1)
            # comp 1 (t): E from trl rows (b=3), S from tim rows (b=2)
            comp_es = [(0, 1), (3, 2)]
            with tc.tile_pool(name="s2", bufs=1) as s2, \
                 tc.tile_pool(name="zp", bufs=1) as zp, \
                 tc.tile_pool(name="zr", bufs=2) as zr, \
                 tc.tile_pool(name="ob", bufs=4) as ob, \
                 tc.tile_pool(name="ps2tp", bufs=2, space="PSUM") as ps2tp, \
                 tc.tile_pool(name="ps2po", bufs=2, space="PSUM") as ps2po:
                ct = s2.tile([128, 2, LCH, 368], mybir.dt.bfloat16, tag="ct")
                nc.sync.dma_start(out=ct[:], in_=ctab[:])
                # bf16 accumulators: [n-part, comp, E/S, nu, kc]
                acc = s2.tile([128, 2, 2, 3, 1472], mybir.dt.bfloat16,
                              tag="acc")

                for ci, (ms, mw) in enumerate(MSEG):
                    mcnt = NC * mw
                    ztmp = zp.tile([128, NC, mw, KC], mybir.dt.bfloat16,
                                   name=f"ztmp{ci}")
                    nc.scalar.dma_start(
                        out=ztmp[:],
                        in_=a2a_out[ci].rearrange("b c m k -> c b m k"))
                    zsrc = ztmp.rearrange("c b m k -> c (b m) k")

                    # transpose c2->m via matmuls against the identity;
                    # 4 transposes per psum bank, one wide eviction copy
                    t = zr.tile([128, 4, NG, 4, 32], mybir.dt.bfloat16,
                                tag="ztr")
                    nc.vector.memset(t[:, :, NG-1, 2:4, :], 0.0)
                    for g in range(NG):
                        kw = min(4, KC - g * 4)
                        ptb = ps2tp.tile([128, 512], mybir.dt.float32,
                                         tag="ptb")
                        for kk in range(kw):
                            nc.tensor.matmul(
                                out=ptb[:mcnt, kk*128:(kk+1)*128],
                                lhsT=zsrc[:, :mcnt, g*4 + kk],
                                rhs=ident[:], start=True, stop=True)
                        pv = ptb[:mcnt, 0:kw*128].rearrange(
                            "m (k b c) -> m b k c", b=4, c=32)
                        if g % 2 == 0:
                            nc.vector.tensor_copy(out=t[:mcnt, :, g, 0:kw, :],
                                                  in_=pv)
                        else:
                            nc.scalar.activation(out=t[:mcnt, :, g, 0:kw, :],
                                                 in_=pv, func=ACT_COPY)

                    # operand-swapped irfft: C-table chunks are the
                    # stationary weights (their 128-row LDWEIGHTS hides
                    # under 512-col z streams); output is n-partitioned.
                    tv = t.rearrange("p b g kk c -> p b (g kk c)")
                    for comp in range(2):
                        bbE, bbS = comp_es[comp]
                        for nu in range(3):
                            pw = min(128, 368 - nu * 128)
                            nsl = slice(nu * 128, nu * 128 + pw)
                            # every chunk: one E pass + one S pass
                            potE = ps2po.tile([128, 1536],
                                              mybir.dt.float32,
                                              tag="pot")
                            potS = ps2po.tile([128, 1536],
                                              mybir.dt.float32,
                                              tag="pot")
                            for pot, reim, bb in ((potE, 0, bbE),
                                                  (potS, 1, bbS)):
                                for r0 in (0, 512, 1024):
                                    r1 = min(r0 + 512, 1472)
                                    nc.tensor.matmul(
                                        out=pot[:pw, r0:r1],
                                        lhsT=ct[:mcnt, reim, ci, nsl],
                                        rhs=tv[:mcnt, bb, r0:r1],
                                        start=True, stop=True)
                            aE = acc[:, comp, 0, nu]
                            aS = acc[:, comp, 1, nu]
                            if ci == 0:
                                nc.vector.tensor_copy(
                                    out=aE[:pw, 0:736],
                                    in_=potE[:pw, 0:736])
                                nc.scalar.activation(
                                    out=aE[:pw, 736:1472],
                                    in_=potE[:pw, 736:1472],
                                    func=ACT_COPY)
                                nc.vector.tensor_copy(
                                    out=aS[:pw, 0:736],
                                    in_=potS[:pw, 0:736])
                                nc.scalar.activation(
                                    out=aS[:pw, 736:1472],
                                    in_=potS[:pw, 736:1472],
                                    func=ACT_COPY)
                            else:
                                nc.vector.tensor_tensor(
                                    aE[:pw], potE[:pw, 0:1472],
                                    aE[:pw], mybir.AluOpType.add)
                                nc.vector.tensor_tensor(
                                    aS[:pw], potS[:pw, 0:1472],
                                    aS[:pw], mybir.AluOpType.add)
                            if ci == 2:
                                # butterfly combine rides DVE (lo) and
                                # gpsimd (hi) while the PE streams the
                                # next (comp, nu) pair
                                osbL = ob.tile([128, 1472],
                                               mybir.dt.bfloat16,
                                               tag="osb")
                                osbH = ob.tile([128, 1472],
                                               mybir.dt.bfloat16,
                                               tag="osb")
                                nc.vector.tensor_tensor(
                                    osbL[:pw], aE[:pw], aS[:pw],
                                    mybir.AluOpType.add)
                                heng = (nc.vector
                                        if (comp == 1 and nu == 2)
                                        else nc.gpsimd)
                                heng.tensor_tensor(
                                    osbH[:pw], aE[:pw], aS[:pw],
                                    mybir.AluOpType.subtract)
                                deng = (nc.sync if nu % 2 == 0
                                        else nc.scalar)
                                deng.dma_start(
                                    out=outsh[0, comp, nu][0:pw],
                                    in_=osbL[:pw])
                                deng = (nc.sync if nu % 2 == 1
                                        else nc.scalar)
                                deng.dma_start(
                                    out=outsh[1, comp, nu][0:pw],
                                    in_=osbH[:pw])
    nc.compile()
    return nc


def _m_perm():
    """Row order of the m axis as seen by stage 2 (segment-major)."""
    perm = []
    for (ms, mw) in MSEG:
        perm += [mb * MC + ms + ml for mb in range(NC) for ml in range(mw)]
    return np.array(perm)


def _host_prep(x_re, x_im, d0, d1):
    xr0, xr1 = x_re[0, :, 0], x_re[0, :, 1]   # (32, L, M)
    xi0, xi1 = x_im[0, :, 0], x_im[0, :, 1]

    # X1 pack only; X2 is built on-chip from X1
    x = np.concatenate([xr0, xi0, -xi1, -xr1], axis=0)   # (128, L, M)
    x = np.transpose(x, (2, 1, 0))                       # (M, L, 128)
    xp = np.zeros((MP, LP, 128), BF16)
    xp[:M, :L] = x
    xv = xp.reshape(NC, MC, LCH, 128, 128)               # (i, ml, lc, p, c)
    xv = np.ascontiguousarray(xv.transpose(0, 3, 1, 2, 4))

    def mkd(d):
        dp = np.zeros((MP, LP, KPP), BF16)
        dp[:M, :L, :K] = np.transpose(d, (0, 2, 1))
        return dp
    D0, D1 = mkd(d0), mkd(d1)
    dv = np.stack([D0, D1], axis=1)                   # (MP, 2, LP, KPP)
    dv = dv.reshape(NC, MC, 2, LCH, 128, KPP)
    dv = np.ascontiguousarray(dv.transpose(0, 4, 1, 2, 3, 5))

    m = np.arange(MP, dtype=np.float64)[:, None]
    n = np.arange(368, dtype=np.float64)[None, :]
    th = 2.0 * np.pi * (m * n) / N
    w = np.full((MP, 1), 2.0); w[0] = 1.0; w[360] = 1.0; w[361:] = 0.0
    Cre = (w * np.cos(th)).astype(np.float32)
    Cim = (-w * np.sin(th)).astype(np.float32)
    Cim[0] = 0.0; Cim[360] = 0.0; Cim[361:] = 0.0
    cv = np.stack([Cre, Cim], axis=1)                 # (MP, 2, 368)
    # chunk-aligned layout: each segment's rows start at a 128-row slot
    cvc = np.zeros((LCH, 128, 2, 368), np.float32)
    for s, (ms, mw) in enumerate(MSEG):
        rows = np.array([mb * MC + ms + ml
                         for mb in range(NC) for ml in range(mw)])
        cvc[s, :len(rows)] = cv[rows]
    cv = np.ascontiguousarray(cvc.transpose(1, 2, 0, 3)).astype(BF16)
    return xv, dv, cv


def kernel(x_re, x_im, d0, d1):
    if "nc" not in _CACHE:
        _CACHE["nc"] = _build()
    nc = _CACHE["nc"]

    xv, dv, cv = _host_prep(np.asarray(x_re), np.asarray(x_im),
                            np.asarray(d0), np.asarray(d1))
    in_maps = [{"xsh": xv[i], "dsh": dv[i], "ctab": cv} for i in range(NC)]
    res = run_bass_kernel_spmd(nc, in_maps, list(range(NC)))

    out = np.empty((B, C, 2, K, N), np.float32)
    for i in range(NC):
        k0 = i * KC
        kv = min(K, k0 + KC) - k0
        # [lohi, comp, nu, p, kc] -> [c, comp, k, lohi, n'=nu*128+p]
        o = res.results[i]["outsh"].astype(np.float32)
        o = o.reshape(2, 2, 3, 128, KC, C)
        o = o.transpose(5, 1, 4, 0, 2, 3).reshape(C, 2, KC, 2, 384)
        lo = o[:, :, :kv, 0, :]
        hi = o[:, :, :kv, 1, :]
        out[0, :, :, k0:k0+kv, 0:361] = lo[..., 0:361]
        out[0, :, :, k0:k0+kv, 361:720] = hi[..., 1:360][..., ::-1]
    return out



# revision 32
# speedup vs baseline: 1.3260x; 1.0300x over previous
"""Distributed inverse real vector SHT on 8 Trainium2 NeuronCores.

Decomposition (2D polar x azimuth, per the original model's parallelism):
  Stage 1 (sharded over m): for each m, the four Legendre contractions are
  two accumulating matmuls  Z[m] = X1[m]^T @ dT0[m] + X2[m]^T @ dT1[m]
  where the 128 columns of X1/X2 pack the four (re/im x s/t) input blocks
  with signs arranged so the PSUM accumulation directly produces
  rows [srl, sim, tim, trl].  Only X1 is loaded from HBM; X2 is a signed
  block-permutation of X1 built on-chip (halves the x traffic).
  All-to-all (3 segments along m, fired as stage 1 completes each).
  Stage 2 (sharded over k): per segment chunk, transpose Z on the PE
  (c2 partition -> m partition), then run the irfft matmuls against
  precomputed cos/sin tables; chunks accumulate into an SBUF f32
  accumulator as their collectives land, so only the last chunk's
  matmuls and a fused add-evict trail the final collective.
"""
import sys
import os
sys.path.insert(0, '/opt/trn_rl_repo')
import numpy as np
import ml_dtypes

from concourse import bacc, tile, mybir, masks
from concourse.bass_utils import run_bass_kernel_spmd

B, C, L, M, K, N = 1, 32, 361, 361, 361, 720
NC = 8
MP = 368                    # m padded to 8*46
MC = MP // NC               # 46 m's per core
MSEG = [(0, 14), (14, 16), (30, 16)]   # 3 a2a segments = stage-2 chunks
KC = 46                     # k's per core
KPP = 368
LP = 384                    # l padded to 3*128
LCH = 3
NG = (KC + 3) // 4          # 12 kj-groups (last has 2)
BF16 = ml_dtypes.bfloat16
ACT_COPY = mybir.ActivationFunctionType.Copy

_CACHE = {}


def _build():
    nc = bacc.Bacc("TRN2", target_bir_lowering=False, debug=False,
                   num_devices=NC)
    xsh = nc.dram_tensor("xsh", [128, MC, LCH, 128], mybir.dt.bfloat16,
                         kind="ExternalInput")
    dsh = nc.dram_tensor("dsh", [128, MC, 2, LCH, KPP], mybir.dt.bfloat16,
                         kind="ExternalInput")
    # butterfly tables: slot 0 = w*cos, 1 = -w*sin, over n' = 0..367
    # (out[n'] = E+S, out[720-n'] = E-S)
    ctab = nc.dram_tensor("ctab", [128, 2, LCH, 368], mybir.dt.bfloat16,
                          kind="ExternalInput")
    outsh = nc.dram_tensor("outsh", [2, 2, 3, 128, 1472], mybir.dt.bfloat16,
                           kind="ExternalOutput")

    m_blocks = [(0, 8), (8, 6), (14, 8), (22, 8), (30, 8), (38, 8)]

    with tile.TileContext(nc) as tc:
        with tc.tile_pool(name="dram", bufs=1, space="DRAM") as dram, \
             tc.tile_pool(name="const", bufs=1) as constp:
            a2a_in = [dram.tile([NC, 128, mw, KC], mybir.dt.bfloat16,
                                name=f"a2ain{s}")
                      for s, (ms, mw) in enumerate(MSEG)]
            a2a_out = [dram.tile([NC, 128, mw, KC], mybir.dt.bfloat16,
                                 name=f"a2aout{s}")
                       for s, (ms, mw) in enumerate(MSEG)]
            ident = constp.tile([128, 128], mybir.dt.bfloat16)
            masks.make_identity(nc, ident[:])

            # ---------------- stage 1: Legendre contractions (m-sharded)
            with tc.tile_pool(name="s1", bufs=2) as s1, \
                 tc.tile_pool(name="zs", bufs=1) as zs, \
                 tc.tile_pool(name="ps1", bufs=4, space="PSUM") as ps1:
                zst = [zs.tile([128, NC, mw, KC], mybir.dt.bfloat16,
                               name=f"zst{s}")
                       for s, (ms, mw) in enumerate(MSEG)]
                for (m0, cnt) in m_blocks:
                    dt = s1.tile([128, 8, 2, LCH, KPP], mybir.dt.bfloat16,
                                 tag="dt")
                    x1 = s1.tile([128, 8, LCH, 128], mybir.dt.bfloat16,
                                 tag="x1")
                    x2 = s1.tile([128, 8, LCH, 128], mybir.dt.bfloat16,
                                 tag="x2")
                    nc.sync.dma_start(out=dt[:, :cnt], in_=dsh[:, m0:m0+cnt])
                    nc.sync.dma_start(out=x1[:, :cnt], in_=xsh[:, m0:m0+cnt])
                    # X2 = [X1 blk2, -X1 blk3, X1 blk0, -X1 blk1]
                    nc.scalar.activation(out=x2[:, :cnt, :, 0:32],
                                         in_=x1[:, :cnt, :, 64:96],
                                         func=ACT_COPY)
                    nc.vector.tensor_scalar_mul(x2[:, :cnt, :, 32:64],
                                                x1[:, :cnt, :, 96:128], -1.0)
                    nc.scalar.activation(out=x2[:, :cnt, :, 64:96],
                                         in_=x1[:, :cnt, :, 0:32],
                                         func=ACT_COPY)
                    nc.vector.tensor_scalar_mul(x2[:, :cnt, :, 96:128],
                                                x1[:, :cnt, :, 32:64], -1.0)
                    xw = (x1, x2)
                    for ml in range(cnt):
                        zt = ps1.tile([128, KPP], mybir.dt.float32, tag="zt")
                        for lc in range(LCH):
                            for w in range(2):
                                nc.tensor.matmul(
                                    out=zt[:],
                                    lhsT=xw[w][:, ml, lc, :],
                                    rhs=dt[:, ml, w, lc, :],
                                    start=(lc == 0 and w == 0),
                                    stop=(lc == LCH - 1 and w == 1),
                                )
                        mg = m0 + ml
                        seg = next(s for s, (ms, mw) in enumerate(MSEG)
                                   if ms <= mg < ms + mw)
                        zv = zt[:].rearrange("p (g k) -> p g k", k=KC)
                        dst = zst[seg][:, :, mg - MSEG[seg][0], :]
                        if mg % 2 == 0:
                            nc.vector.tensor_copy(out=dst, in_=zv)
                        else:
                            nc.scalar.activation(out=dst, in_=zv,
                                                 func=ACT_COPY)
                    # fire each segment's a2a writes as soon as complete
                    # (seg 2 rides the sync ring, idle once input is done)
                    for s, (ms, mw) in enumerate(MSEG):
                        if m0 + cnt == ms + mw:
                            weng = nc.sync if s == 2 else nc.scalar
                            for kg in range(NC):
                                weng.dma_start(
                                    out=a2a_in[s][kg], in_=zst[s][:, kg])

            for s in range(len(MSEG)):
                nc.gpsimd.collective_compute(
                    "AllToAll", mybir.AluOpType.bypass,
                    replica_groups=[list(range(NC))],
                    ins=[a2a_in[s].opt()], outs=[a2a_out[s].opt()],
                )

            # ---------------- stage 2: irfft as matmul (k-sharded)
            # Butterfly: E = Cre^T zre, S = Cim^T zim over n' = 0..360;
            # out[n'] = E + S, out[720-n'] = E - S.
            # comp 0 (s): E from srl rows (b=0), S from sim rows (b=1)
            # comp 1 (t): E from trl rows (b=3), S from tim rows (b=2)
            comp_es = [(0, 1), (3, 2)]
            with tc.tile_pool(name="s2", bufs=1) as s2, \
                 tc.tile_pool(name="zp", bufs=1) as zp, \
                 tc.tile_pool(name="zr", bufs=2) as zr, \
                 tc.tile_pool(name="ob", bufs=4) as ob, \
                 tc.tile_pool(name="ps2tp", bufs=2, space="PSUM") as ps2tp, \
                 tc.tile_pool(name="ps2po", bufs=2, space="PSUM") as ps2po:
                ct = s2.tile([128, 2, LCH, 368], mybir.dt.bfloat16, tag="ct")
                nc.sync.dma_start(out=ct[:], in_=ctab[:])
                # bf16 accumulators: [n-part, comp, E/S, nu, kc]
                acc = s2.tile([128, 2, 2, 3, 1472], mybir.dt.bfloat16,
                              tag="acc")

                for ci, (ms, mw) in enumerate(MSEG):
                    mcnt = NC * mw
                    ztmp = zp.tile([128, NC, mw, KC], mybir.dt.bfloat16,
                                   name=f"ztmp{ci}")
                    nc.scalar.dma_start(
                        out=ztmp[:],
                        in_=a2a_out[ci].rearrange("b c m k -> c b m k"))
                    zsrc = ztmp.rearrange("c b m k -> c (b m) k")

                    # transpose c2->m via matmuls against the identity;
                    # 4 transposes per psum bank, one wide eviction copy
                    t = zr.tile([128, 4, NG, 4, 32], mybir.dt.bfloat16,
                                tag="ztr")
                    nc.vector.memset(t[:, :, NG-1, 2:4, :], 0.0)
                    for g in range(NG):
                        kw = min(4, KC - g * 4)
                        ptb = ps2tp.tile([128, 512], mybir.dt.float32,
                                         tag="ptb")
                        for kk in range(kw):
                            nc.tensor.matmul(
                                out=ptb[:mcnt, kk*128:(kk+1)*128],
                                lhsT=zsrc[:, :mcnt, g*4 + kk],
                                rhs=ident[:], start=True, stop=True)
                        pv = ptb[:mcnt, 0:kw*128].rearrange(
                            "m (k b c) -> m b k c", b=4, c=32)
                        if g % 2 == 0:
                            nc.vector.tensor_copy(out=t[:mcnt, :, g, 0:kw, :],
                                                  in_=pv)
                        else:
                            nc.scalar.activation(out=t[:mcnt, :, g, 0:kw, :],
                                                 in_=pv, func=ACT_COPY)

                    # operand-swapped irfft: C-table chunks are the
                    # stationary weights (their 128-row LDWEIGHTS hides
                    # under 512-col z streams); output is n-partitioned.
                    tv = t.rearrange("p b g kk c -> p b (g kk c)")
                    for comp in range(2):
                        bbE, bbS = comp_es[comp]
                        for nu in range(3):
                            pw = min(128, 368 - nu * 128)
                            nsl = slice(nu * 128, nu * 128 + pw)
                            # every chunk: one E pass + one S pass
                            potE = ps2po.tile([128, 1536],
                                              mybir.dt.float32,
                                              tag="pot")
                            potS = ps2po.tile([128, 1536],
                                              mybir.dt.float32,
                                              tag="pot")
                            for pot, reim, bb in ((potE, 0, bbE),
                                                  (potS, 1, bbS)):
                                for r0 in (0, 512, 1024):
                                    r1 = min(r0 + 512, 1472)
                                    nc.tensor.matmul(
                                        out=pot[:pw, r0:r1],
                                        lhsT=ct[:mcnt, reim, ci, nsl],
                                        rhs=tv[:mcnt, bb, r0:r1],
                                        start=True, stop=True)
                            aE = acc[:, comp, 0, nu]
                            aS = acc[:, comp, 1, nu]
                            if ci == 0:
                                nc.vector.tensor_copy(
                                    out=aE[:pw, 0:736],
                                    in_=potE[:pw, 0:736])
                                nc.scalar.activation(
                                    out=aE[:pw, 736:1472],
                                    in_=potE[:pw, 736:1472],
                                    func=ACT_COPY)
                                nc.vector.tensor_copy(
                                    out=aS[:pw, 0:736],
                                    in_=potS[:pw, 0:736])
                                nc.scalar.activation(
                                    out=aS[:pw, 736:1472],
                                    in_=potS[:pw, 736:1472],
                                    func=ACT_COPY)
                            else:
                                nc.vector.tensor_tensor(
                                    aE[:pw], potE[:pw, 0:1472],
                                    aE[:pw], mybir.AluOpType.add)
                                nc.vector.tensor_tensor(
                                    aS[:pw], potS[:pw, 0:1472],
                                    aS[:pw], mybir.AluOpType.add)
                            if ci == 2:
                                # butterfly combine rides DVE (lo) and
                                # gpsimd (hi) while the PE streams the
                                # next (comp, nu) pair
                                osbL = ob.tile([128, 1472],
                                               mybir.dt.bfloat16,
                                               tag="osb")
                                osbH = ob.tile([128, 1472],
                                               mybir.dt.bfloat16,
                                               tag="osb")
                                nc.vector.tensor_tensor(
                                    osbL[:pw], aE[:pw], aS[:pw],
                                    mybir.AluOpType.add)
                                nc.gpsimd.tensor_tensor(
                                    osbH[:pw], aE[:pw], aS[:pw],
                                    mybir.AluOpType.subtract)
                                deng = (nc.sync if nu % 2 == 0
                                        else nc.scalar)
                                deng.dma_start(
                                    out=outsh[0, comp, nu][0:pw],
                                    in_=osbL[:pw])
                                deng = (nc.sync if nu % 2 == 1
                                        else nc.scalar)
                                deng.dma_start(
                                    out=outsh[1, comp, nu][0:pw],
                                    in_=osbH[:pw])
    nc.compile()
    return nc


def _m_perm():
    """Row order of the m axis as seen by stage 2 (segment-major)."""
    perm = []
    for (ms, mw) in MSEG:
        perm += [mb * MC + ms + ml for mb in range(NC) for ml in range(mw)]
    return np.array(perm)


def _host_prep(x_re, x_im, d0, d1):
    xr0, xr1 = x_re[0, :, 0], x_re[0, :, 1]   # (32, L, M)
    xi0, xi1 = x_im[0, :, 0], x_im[0, :, 1]

    # X1 pack only; X2 is built on-chip from X1
    x = np.concatenate([xr0, xi0, -xi1, -xr1], axis=0)   # (128, L, M)
    x = np.transpose(x, (2, 1, 0))                       # (M, L, 128)
    xp = np.zeros((MP, LP, 128), BF16)
    xp[:M, :L] = x
    xv = xp.reshape(NC, MC, LCH, 128, 128)               # (i, ml, lc, p, c)
    xv = np.ascontiguousarray(xv.transpose(0, 3, 1, 2, 4))

    def mkd(d):
        dp = np.zeros((MP, LP, KPP), BF16)
        dp[:M, :L, :K] = np.transpose(d, (0, 2, 1))
        return dp
    D0, D1 = mkd(d0), mkd(d1)
    dv = np.stack([D0, D1], axis=1)                   # (MP, 2, LP, KPP)
    dv = dv.reshape(NC, MC, 2, LCH, 128, KPP)
    dv = np.ascontiguousarray(dv.transpose(0, 4, 1, 2, 3, 5))

    m = np.arange(MP, dtype=np.float64)[:, None]
    n = np.arange(368, dtype=np.float64)[None, :]
    th = 2.0 * np.pi * (m * n) / N
    w = np.full((MP, 1), 2.0); w[0] = 1.0; w[360] = 1.0; w[361:] = 0.0
    Cre = (w * np.cos(th)).astype(np.float32)
    Cim = (-w * np.sin(th)).astype(np.float32)
    Cim[0] = 0.0; Cim[360] = 0.0; Cim[361:] = 0.0
    cv = np.stack([Cre, Cim], axis=1)                 # (MP, 2, 368)
    # chunk-aligned layout: each segment's rows start at a 128-row slot
    cvc = np.zeros((LCH, 128, 2, 368), np.float32)
    for s, (ms, mw) in enumerate(MSEG):
        rows = np.array([mb * MC + ms + ml
                         for mb in range(NC) for ml in range(mw)])
        cvc[s, :len(rows)] = cv[rows]
    cv = np.ascontiguousarray(cvc.transpose(1, 2, 0, 3)).astype(BF16)
    return xv, dv, cv


def kernel(x_re, x_im, d0, d1):
    if "nc" not in _CACHE:
        _CACHE["nc"] = _build()
    nc = _CACHE["nc"]

    xv, dv, cv = _host_prep(np.asarray(x_re), np.asarray(x_im),
                            np.asarray(d0), np.asarray(d1))
    in_maps = [{"xsh": xv[i], "dsh": dv[i], "ctab": cv} for i in range(NC)]
    res = run_bass_kernel_spmd(nc, in_maps, list(range(NC)))

    out = np.empty((B, C, 2, K, N), np.float32)
    for i in range(NC):
        k0 = i * KC
        kv = min(K, k0 + KC) - k0
        # [lohi, comp, nu, p, kc] -> [c, comp, k, lohi, n'=nu*128+p]
        o = res.results[i]["outsh"].astype(np.float32)
        o = o.reshape(2, 2, 3, 128, KC, C)
        o = o.transpose(5, 1, 4, 0, 2, 3).reshape(C, 2, KC, 2, 384)
        lo = o[:, :, :kv, 0, :]
        hi = o[:, :, :kv, 1, :]
        out[0, :, :, k0:k0+kv, 0:361] = lo[..., 0:361]
        out[0, :, :, k0:k0+kv, 361:720] = hi[..., 1:360][..., ::-1]
    return out

